# revision 63
# baseline (speedup 1.0000x reference)
"""Trainium2 Bass kernel for nn_Predictor_67585605370461 (segment_reduce).

Per patch (N=4194304, 9 elements each):
  m = edg > 0.5 ; md = mean(img | m), ma = mean(img | ~m)
  out = valid ? (md > ma ? 0 : 1) : 2   (valid iff 0 < count(m) < 9)
Global: avgB/avgW = masked means of center pixels over out==0 / out==1
  corr = out==2 ? (|v-avgB| < |v-avgW| ? 0 : 1) : out  -> reshape [2048,2048]

Key algebra used by this kernel (one fused mask pass + two 9-reduces):
  vp16 = v + 16 ;  q = (e > 0.5) * vp16
  T = sum9(vp16) = sall + 144 ;  S = sum9(q) = sd + 16*c
  c  = roundNE(S/16)            (exact: sd < c <= 8 for valid patches)
  white  <=>  9*sd <= c*sall  <=>  9*S <= c*T     (exact algebra)
  c == 0  <=>  S < 16 ;  c == 9  <=>  S >= 144    (exact thresholds)
  pvb = (9S <= c*T) max (S >= 144)   ("not black": white or unknown)
  nv = (S < 16) + (S >= 144)    (unknown indicator)
  corr = pvb - nv * ((v4*s + b2) < 0),  s = avgW-avgB, b2 = -mid*s

Engine split per tile keeps every engine under the HBM stream rate:
  ACT: +16 bias pass, center extract, phase-2 affine
  DVE: q (fused select), S reduce, small per-patch ops
  Pool: T via tree-adds, c*T, unknown indicator, final add (u8 out)
  PE:  cross-partition reduction + broadcast of the 4 global scalars

The two global means avgB/avgW differ by only ~8e-6 on this data, and
their SIGN decides ~16k unknown pixels, so the masked sums must cover
every patch exactly and be AllReduced across all 8 cores (any sampled or
per-shard estimate flips the sign at random).  Device pitfalls found on
real TRN2: ACT-table ops are interpolated (never use them where exact
f32 compares matter), and sums of identical values in different add
orders (S vs T at c==9) differ by ulps, so that case is forced
explicitly.
"""

import numpy as np

import concourse.bass as bass
import concourse.bacc as bacc
import concourse.mybir as mybir
import concourse.tile as tile
from concourse import bass_utils

N_CORES = 8
N_PATCH = 4194304
NP_CORE = N_PATCH // N_CORES  # 524288
P = 128
NINE = 9
NPW = NP_CORE // P            # 4096 patches per partition per core
# tapered tile widths: small at the start (fast pipeline fill) and at the
# end (short drain chain after the last input DMA lands)
WIDTHS = [128, 128] + [256] * 14 + [128, 128]
assert sum(WIDTHS) == NPW
OFFS = [sum(WIDTHS[:i]) for i in range(len(WIDTHS))]
# phase-2 chunk boundaries (tile-aligned, tapered at the end)
CHUNKS = [(0, 1024), (1024, 1024), (2048, 1024), (3072, 512),
          (3584, 256), (3840, 128), (3968, 128)]
H_OUT = 2048
RND = 12582912.0              # 1.5 * 2**23: float32 round-to-nearest trick
PRIO_STREAM = 40              # stream ops jump ~1.5 tiles ahead, not globally

f32 = mybir.dt.float32
u8 = mybir.dt.uint8
Alu = mybir.AluOpType
Act = mybir.ActivationFunctionType
X = mybir.AxisListType.X

_CACHE = {}


def _build(stub_cc=False):
    num_devices = 1 if stub_cc else N_CORES
    nc = bacc.Bacc("TRN2", target_bir_lowering=False, debug=False,
                   num_devices=num_devices)
    img = nc.dram_tensor("img", [NP_CORE, NINE], f32, kind="ExternalInput")
    edg = nc.dram_tensor("edg", [NP_CORE, NINE], f32, kind="ExternalInput")
    out = nc.dram_tensor("out", [NP_CORE], f32, kind="ExternalOutput")

    # partition-contiguous layout: partition p owns patches [p*NPW,(p+1)*NPW)
    # of this core's shard; tile i covers columns [o_i, o_i+w_i) of every
    # partition, so each DMA moves 128 runs of w_i*9 contiguous floats.
    img_f = img.ap().rearrange("(p j) n -> p (j n)", p=P)
    edg_f = edg.ap().rearrange("(p j) n -> p (j n)", p=P)
    out_f = out.ap().rearrange("(p j) -> p j", p=P)

    with tile.TileContext(nc) as tc:
        with (
            tc.tile_pool(name="vin", bufs=3) as vpool,
            tc.tile_pool(name="ein", bufs=3) as epool,
            tc.tile_pool(name="lt", bufs=2) as lpool,
            tc.tile_pool(name="small", bufs=2) as spool,
            tc.tile_pool(name="persist", bufs=1) as pers,
            tc.tile_pool(name="psum", bufs=1, space="PSUM") as psum,
            tc.tile_pool(name="dram", bufs=1, space="DRAM") as dram,
        ):
            # per-patch persists for the deferred phase 2
            pv4 = pers.tile([P, NPW], f32)
            pnv = pers.tile([P, NPW], f32)
            pvw = pers.tile([P, NPW], f32)
            po = pers.tile([P, NPW], f32)
            # per-tile accumulators: [pvb, pvb*v4, nv, nv*v4, v4]
            accs = pers.tile([P, 5, len(WIDTHS)], f32)
            p16 = pers.tile([P, 1], f32)
            nc.vector.memset(p16[:], 16.0)
            n16 = pers.tile([P, 1], f32)
            nc.vector.memset(n16[:], -16.0)
            n80 = pers.tile([P, 1], f32)
            nc.vector.memset(n80[:], -80.0)
            prnd = pers.tile([P, 1], f32)
            nc.vector.memset(prnd[:], RND)
            nrnd = pers.tile([P, 1], f32)
            nc.vector.memset(nrnd[:], -RND)
            onesP = pers.tile([P, 1], f32)
            nc.vector.memset(onesP[:], 1.0)
            ones1 = pers.tile([1, P], f32)
            nc.vector.memset(ones1[:], 1.0)
            gmsh = pers.tile([P, 1024], f32)
            # prewarm the ACT function table during the first input DMA so
            # the first vp16 doesn't eat the 1.3us LoadActFuncSet latency
            warm = pers.tile([P, 1], f32)
            with tc.high_priority(offset=200000):
                nc.scalar.activation(warm[:], onesP[:], Act.Abs, bias=p16[:])

            for i, (off, w) in enumerate(zip(OFFS, WIDTHS)):
                sl = slice(off, off + w)
                # stream-critical ops (fed directly by the DMA stream) get a
                # globally earlier priority class so no engine queues a
                # cross-engine-dependent tail op ahead of them
                with tc.high_priority(offset=PRIO_STREAM):
                    Vt = vpool.tile([P, w * NINE], f32, tag=f"V{w}",
                                    bufs={256: 2, 128: 3}.get(w, 6))
                    nc.sync.dma_start(Vt[:], img_f[:, off * NINE:(off + w) * NINE])
                    Et = epool.tile([P, w * NINE], f32, tag=f"E{w}",
                                    bufs={256: 2, 128: 3}.get(w, 6))
                    nc.sync.dma_start(Et[:], edg_f[:, off * NINE:(off + w) * NINE])

                    v3 = Vt[:].rearrange("p (w n) -> p w n", n=NINE)
                    e3 = Et[:].rearrange("p (w n) -> p w n", n=NINE)

                    # ACT: vp16 = v + 16 (in place), center extract v4 = v
                    nc.scalar.activation(Vt[:], Vt[:], Act.Identity,
                                         bias=p16[:])
                    # exact DVE extract: the ACT Identity table is coarse
                    # near x~16.5 and injects ~1e-2 noise into v4
                    nc.vector.tensor_scalar(pv4[:, sl], v3[:, :, 4],
                                            1.0, -16.0, op0=Alu.mult,
                                            op1=Alu.add)

                    # Pool: T = sum9(vp16) via tree adds
                    l1 = lpool.tile([P, w, 4], f32, tag=f"l1{w}")
                    nc.gpsimd.tensor_tensor(l1[:], v3[:, :, 0:4],
                                            v3[:, :, 4:8], op=Alu.add)
                    nc.gpsimd.tensor_tensor(l1[:, :, 0:2], l1[:, :, 0:2],
                                            l1[:, :, 2:4], op=Alu.add)
                    nc.gpsimd.tensor_tensor(l1[:, :, 0:1], l1[:, :, 0:1],
                                            l1[:, :, 1:2], op=Alu.add)
                    T = spool.tile([P, w], f32, tag=f"T{w}")
                    nc.gpsimd.tensor_tensor(T[:], l1[:, :, 0], v3[:, :, 8],
                                            op=Alu.add)

                    # DVE: q = (e > 0.5) * vp16 (in place), S = sum9(q)
                    nc.vector.scalar_tensor_tensor(Et[:], Et[:], 0.5, Vt[:],
                                                   op0=Alu.is_gt, op1=Alu.mult)
                    S = spool.tile([P, w], f32, tag=f"S{w}")
                    nc.vector.tensor_reduce(S[:], e3, axis=X, op=Alu.add)

                # c = roundNE(S/16) via the 1.5*2^23 trick (exact for
                # valid c) on the lightly-loaded ACT engine
                y1 = spool.tile([P, w], f32, tag=f"y1{w}")
                nc.scalar.activation(y1[:], S[:], Act.Identity,
                                     bias=prnd[:], scale=0.0625)
                cc = spool.tile([P, w], f32, tag=f"cc{w}")
                nc.scalar.activation(cc[:], y1[:], Act.Identity, bias=nrnd[:])

                # u = c*T ; pvb = not-black = (9S <= u).  Self-masked at
                # both invalid ends: c==0 -> S==0 exactly -> true; c==9 ->
                # S<=T always -> true.  So pvb = white01 on valid patches
                # and 1 on unknowns; corr = pvb - nv*((v4-mid)*s < 0).
                u = spool.tile([P, w], f32, tag=f"u{w}")
                nc.gpsimd.tensor_tensor(u[:], cc[:], T[:], op=Alu.mult)
                pvx = spool.tile([P, w], f32, tag=f"pvx{w}")
                nc.vector.scalar_tensor_tensor(pvx[:], S[:], 9.0, u[:],
                                               op0=Alu.mult, op1=Alu.is_le)
                # c==9: S and T are sums of identical values in different
                # add orders, so 9S<=9T is an ulp coin-flip -- force 1
                nc.vector.scalar_tensor_tensor(
                    pvw[:, sl], S[:], 144.0, pvx[:],
                    op0=Alu.is_ge, op1=Alu.max,
                    accum_out=accs[:, 0, i:i + 1])

                # nv = (S < 16) | (S >= 144), exact ALU compares (the ACT
                # table-interpolated Abs smears the boundary and flips ~1.4k
                # pixels)
                t1 = spool.tile([P, w], f32, tag=f"t1{w}")
                nc.vector.tensor_scalar(t1[:], S[:], 16.0, None,
                                        op0=Alu.is_lt)
                nc.vector.scalar_tensor_tensor(
                    pnv[:, sl], S[:], 144.0, t1[:],
                    op0=Alu.is_ge, op1=Alu.add,
                    accum_out=accs[:, 2, i:i + 1])

                # exact global masked sums (the avgW-avgB sign is ~8e-6 and
                # decides ~16k unknown pixels, so nothing may be sampled):
                # Z3 = sum(pvb*v4) on DVE, Z4 = sum(nv*v4) via Pool+ACT
                av = spool.tile([P, w], f32, tag=f"av{w}")
                nc.vector.tensor_scalar(av[:], pv4[:, sl], 1.0, 0.0,
                                        op0=Alu.mult, op1=Alu.add,
                                        accum_out=accs[:, 4, i:i + 1])
                z3 = spool.tile([P, w], f32, tag=f"z3{w}")
                nc.vector.tensor_tensor(z3[:], pvw[:, sl], pv4[:, sl],
                                        op=Alu.mult)
                z3a = spool.tile([P, w], f32, tag=f"z3a{w}")
                nc.vector.tensor_scalar(z3a[:], z3[:], 1.0, 0.0,
                                        op0=Alu.mult, op1=Alu.add,
                                        accum_out=accs[:, 1, i:i + 1])
                z4 = spool.tile([P, w], f32, tag=f"z4{w}")
                nc.gpsimd.tensor_tensor(z4[:], pnv[:, sl], pv4[:, sl],
                                        op=Alu.mult)
                z4a = spool.tile([P, w], f32, tag=f"z4a{w}")
                nc.vector.tensor_scalar(z4a[:], z4[:], 1.0, 0.0,
                                        op0=Alu.mult, op1=Alu.add,
                                        accum_out=accs[:, 3, i:i + 1])

            # ---- global scalars: reduce accs, AllReduce across cores ----
            acc5 = pers.tile([P, 5], f32)
            nc.vector.tensor_reduce(acc5[:], accs[:], axis=X, op=Alu.add)
            # AllReduce the [P,5] partials across cores (baseline recipe),
            # then fold partitions on DVE via a flattened [1, P*5] view
            cc_in = dram.tile([P, 5], f32)
            cc_out = dram.tile([P, 5], f32, addr_space="Shared")
            nc.sync.dma_start(cc_in[:], acc5[:])
            if stub_cc:
                gsrc = cc_in
            else:
                nc.gpsimd.collective_compute(
                    "AllReduce", Alu.add,
                    replica_groups=[list(range(N_CORES))],
                    ins=[cc_in[:].opt()], outs=[cc_out[:].opt()])
                gsrc = cc_out
            gflat = pers.tile([1, P * 5], f32)
            nc.sync.dma_start(
                gflat[:], gsrc[:].rearrange("(o p) q -> o (p q)", o=1))
            g = pers.tile([1, 5], f32)
            nc.vector.tensor_reduce(
                g[:], gflat[:].rearrange("o (p q) -> o q p", q=5),
                axis=X, op=Alu.add)
            # g = [sum_pvb, sum_pvb*v4, sum_nv, sum_nv*v4, sum_v4] (global)
            # cntW = g0-g2, cntB = N-g0, sumW = g1-g3, sumB = g4-g1
            cb = pers.tile([1, 2], f32)  # [max(cntW,1), max(cntB,1)]
            nc.vector.tensor_scalar(cb[:, 0:1], g[:, 0:1], g[:, 2:3], 1.0,
                                    op0=Alu.subtract, op1=Alu.max)
            nc.vector.tensor_scalar(cb[:, 1:2], g[:, 0:1], -1.0,
                                    float(N_PATCH), op0=Alu.mult, op1=Alu.add)
            nc.vector.tensor_scalar(cb[:, 1:2], cb[:, 1:2], 1.0, None,
                                    op0=Alu.max)
            sb = pers.tile([1, 2], f32)  # [sumW, sumB]
            nc.vector.tensor_scalar(sb[:, 0:1], g[:, 1:2], g[:, 3:4], None,
                                    op0=Alu.subtract)
            nc.vector.tensor_scalar(sb[:, 1:2], g[:, 4:5], g[:, 1:2], None,
                                    op0=Alu.subtract)
            rc = pers.tile([1, 2], f32)
            nc.vector.reciprocal(rc[:], cb[:])
            avg = pers.tile([1, 2], f32)  # [avgW, avgB]
            nc.vector.tensor_tensor(avg[:], sb[:], rc[:], op=Alu.mult)
            sc = pers.tile([1, 2], f32)  # [s, b2]
            nc.vector.tensor_scalar(sc[:, 0:1], avg[:, 0:1], avg[:, 1:2],
                                    None, op0=Alu.subtract)
            mid = pers.tile([1, 1], f32)
            nc.vector.tensor_scalar(mid[:], avg[:, 0:1], avg[:, 1:2], -0.5,
                                    op0=Alu.add, op1=Alu.mult)
            nc.vector.tensor_tensor(sc[:, 1:2], mid[:], sc[:, 0:1],
                                    op=Alu.mult)
            # scale (s, b2) by 2^20 (exact): |y| would otherwise be ~1e-6,
            # below the effective precision of the fast DVE path
            nc.vector.tensor_scalar(sc[:], sc[:], 1048576.0, None,
                                    op0=Alu.mult)
            pb = psum.tile([P, 2], f32)
            nc.tensor.matmul(pb[:], ones1[:], sc[:], start=True, stop=True)
            scb = pers.tile([P, 2], f32)
            nc.vector.tensor_copy(scb[:], pb[:])

            # ---- phase 2: corr = pvb - nv*((v4 - mid)*s < 0), chunked ----
            for ci, (c0, cw) in enumerate(CHUNKS):
                ch = slice(c0, c0 + cw)
                # exact DVE multiply-add: |y| can be ~1e-6 and the ACT
                # table-interpolated Identity smears the sign
                y = spool.tile([P, cw], f32, tag=f"y{cw}_{ci}", bufs=1)
                nc.vector.tensor_scalar(y[:], pv4[:, ch], scb[:, 0:1],
                                        scb[:, 1:2], op0=Alu.mult,
                                        op1=Alu.add)
                nc.vector.scalar_tensor_tensor(gmsh[:, 0:cw], y[:], 0.0,
                                               pnv[:, ch],
                                               op0=Alu.is_lt, op1=Alu.mult)
                nc.vector.tensor_tensor(po[:, ch], pvw[:, ch], gmsh[:, 0:cw],
                                        op=Alu.subtract)
                nc.sync.dma_start(out_f[:, ch], po[:, ch])

    nc.compile()
    return nc


def _get_nc():
    if "nc" not in _CACHE:
        _CACHE["nc"] = _build()
    return _CACHE["nc"]


def run(image, edges_prob, gt=None, trace=False, tmpdir=None):
    nc = _get_nc()
    img = np.ascontiguousarray(np.asarray(image), dtype=np.float32)
    edg = np.ascontiguousarray(np.asarray(edges_prob), dtype=np.float32)
    img = img.reshape(N_PATCH, NINE)
    edg = edg.reshape(N_PATCH, NINE)
    in_maps = []
    for c in range(N_CORES):
        sl = slice(c * NP_CORE, (c + 1) * NP_CORE)
        in_maps.append({"img": img[sl], "edg": edg[sl]})
    res = bass_utils.run_bass_kernel_spmd(
        nc, in_maps, core_ids=list(range(N_CORES)),
        trace=trace, tmpdir=tmpdir)
    shards = [res.results[c]["out"] for c in range(N_CORES)]
    full = np.concatenate(shards).astype(np.float32).reshape(H_OUT, H_OUT)
    return full, res


def kernel(image, edges_prob, gt=None, **_ignored):
    full, _ = run(image, edges_prob, gt)
    return full


def _numpy_model(image, edges_prob):
    """Host model of the device algorithm (global f32-ish scalars)."""
    img = np.asarray(image).reshape(N_PATCH, NINE).astype(np.float32)
    edg = np.asarray(edges_prob).reshape(N_PATCH, NINE).astype(np.float32)
    vp16 = (img + np.float32(16.0)).astype(np.float32)
    q = np.where(edg > 0.5, vp16, np.float32(0.0)).astype(np.float32)
    T = vp16.sum(axis=1, dtype=np.float32)
    S = q.sum(axis=1, dtype=np.float32)
    cc = np.float32(S * np.float32(0.0625) + np.float32(RND)) - np.float32(RND)
    u = (cc * T).astype(np.float32)
    pvb = (np.float32(9.0) * S <= u).astype(np.float32)
    nv = (np.abs(S - np.float32(80.0)) >= 64.0).astype(np.float32)
    v4 = vp16[:, 4] - np.float32(16.0)
    g0 = pvb.sum(dtype=np.float64)
    g1 = (pvb * v4).sum(dtype=np.float64)
    g2 = nv.sum(dtype=np.float64)
    g3 = (nv * v4).sum(dtype=np.float64)
    g4 = v4.sum(dtype=np.float64)
    cntW = max(g0 - g2, 1.0)
    cntB = max(N_PATCH - g0, 1.0)
    sumW = g1 - g3
    sumB = g4 - g1
    avgW = sumW / cntW
    avgB = sumB / cntB
    s = np.float32(avgW - avgB)
    b2 = np.float32(-0.5 * (avgW + avgB) * s)
    h = nv * (v4 * s + b2 < 0).astype(np.float32)
    corr = (pvb - h).astype(np.float32)
    return corr.reshape(H_OUT, H_OUT)
